# revision 36
# baseline (speedup 1.0000x reference)
"""Trainium2 Bass kernel for AttentionGuidedConv.

Reference semantics (B=C=96, L=8192, K=31, A=512):
    kernels = attention_weights @ proj_w.T + proj_b          # [96, 31]
    y[b, t, o] = sum_k x[b, t+k, o] * kernels[o, k]          # [96, 8162, 96]

The conv weight depends only on the channel index o, so every batch shares
channel o's kernel.

Strategy (final, HW-measured ~90us vs the 118.7us fp16-wire baseline):
  - The fp16-wire baseline was DMA-byte-bound (37.7MB/core, ~360GB/s
    per-core roofline).  The error metric is max|err|/max|expected|
    (GLOBAL absmax), so uniform int8 quantization is far kinder than
    fp8; int8 needs an on-chip int8->fp16 dequant pass because the PE
    only reads fp32/bf16/fp16/fp8 (no int8 matmul on TRN2).
  - HW-measured engine rates (cost model is optimistic): Pool
    tensor_copy ~2.0ns/free-elem, DVE int8-in ~1.5ns, DVE fp32-in
    ~1.2ns, ACT ~0.54-1.1ns; 4-D strided APs add ~0.5us/instruction
    vs flat [128, N] ones.  Full-int8 input makes dequant + PSUM-evac
    engine time (~85us) exceed the DMA floor it buys (54us), so:
  - MIXED wire: 6 of 12 channels/core ship fp16 straight into SBUF
    (zero dequant), 6 ship int8 (dequant: ch 0-1 Pool, ch 2-5 ACT;
    DVE does none -- it is slowest at int8).  OUTPUT ships int8:
    global y scale sy (host-sampled absmax x1.15), 1/sy folded into
    the band matrices along with the per-channel x scales, so the
    PSUM evac copy converts fp32->int8 directly.  End-to-end rel err
    1.296e-2 on the fixed-seed inputs vs the 2e-2 gate.
  - PSUM tiles are [128, 2ch, 8, 64] (2 banks x 4 bufs = all 8): 4
    accumulating matmuls per tile, one flat evac per tile (DVE groups
    0,1,4,5 / ACT groups 2,3).  4-deep rotation keeps the PE p-state
    at 2.4GHz (215ns per 512-col matmul, measured).
  - Software pipelining: input DMAs + Pool dequant issue one block
    AHEAD of the matmuls; ACT dequant for block b+1 is emitted AFTER
    block b's ACT evacs + out-DMA (in-order engine, so order = fate);
    fp16-direct channel groups matmul first each block; 12 warmup
    matmuls flip the PE clock gate during the DMA preamble; the last
    block's out-DMA is split so the drain tail overlaps the evacs.
  - Channel sharding (12 ch/core), non-overlapping 128-row windows,
    corner-band second matmul (outputs m>=98 borrow rows [0,30) of
    the next window), zero-padded full-height stationary, contiguous
    host-side relayout, sync-ring in / scalar-ring out DMA inherited
    from the baseline (kernel_baseline_v0.py.bak).

Per-core traffic: 14.2MB in (6ch fp16 + 6ch int8 + bands) + 9.44MB out
int8 = ~24MB -> ~64us DMA busy; DVE ~58us, ACT ~50us, Pool ~44us; PE
~66us busy at 2.4GHz -- the per-block critical path is PE (5.2us) +
~0.45us block-boundary resync, plus ~8us start and ~7us drain.
"""

import os

import numpy as np

import concourse.bass as bass
import concourse.bacc as bacc
import concourse.mybir as mybir
import concourse.tile as tile
from concourse.bass_utils import run_bass_kernel_spmd

F32 = mybir.dt.float32
F16 = mybir.dt.float16
I8 = mybir.dt.int8

B, L, C = 96, 8192, 96
K = 31
A = 512
N_CORES = 8
C_SHARD = C // N_CORES          # 12 channels per core
WIN = 128                       # window rows == outputs per chunk (no overlap)
NW = L // WIN                   # 64 windows
OVER = K - 1                    # 30 rows borrowed from the next window
L_OUT = L - K + 1               # 8162
CG = int(os.environ.get("KERNEL_CG", "2"))  # channels per evac chunk & psum tile

S_BLK = int(os.environ.get("KERNEL_S_BLK", "8"))      # batches per block
N_BLK = B // S_BLK
F16CH = int(os.environ.get("KERNEL_F16CH", "6"))      # fp16-direct ch/core
I8CH = C_SHARD - F16CH                                # int8 channels/core
XQ_BUFS = int(os.environ.get("KERNEL_XQ_BUFS", "5"))  # int8 staging tiles
XH_BUFS = int(os.environ.get("KERNEL_XH_BUFS", "5"))  # fp16 input tiles
OUT_BUFS = int(os.environ.get("KERNEL_OUT_BUFS", "5"))
PSUM_BUFS = int(os.environ.get("KERNEL_PSUM_BUFS", "4"))
N_WARM = int(os.environ.get("KERNEL_N_WARM", "12"))    # PE warm-up matmuls
# sync | scalar: ring for the one-shot band-matrix loads
BANDS_RING = os.environ.get("KERNEL_BANDS_RING", "scalar")
# dequant engine per int8 channel (p=Pool, d=DVE, a=ACT), len == I8CH;
# adjacent same-letter channels merge into one instruction
DEQ_MAP = os.environ.get("KERNEL_DEQ_MAP", "ppaaaa")
# evac engine per 3-ch chunk index 0..3    (d=DVE, a=ACT)
EVAC_MAP = os.environ.get("KERNEL_EVAC_MAP", "ddaadd")
# output int8 absmax safety inflation over the sampled estimate
SY_INFLATE = float(os.environ.get("KERNEL_SY_INFLATE", "1.15"))


def build_nc(s_blk: int = S_BLK) -> bass.Bass:
    n_blk = B // s_blk
    free = C_SHARD * s_blk * NW
    f16_free = F16CH * s_blk * NW
    i8_free = I8CH * s_blk * NW
    nc = bacc.Bacc(None, target_bir_lowering=False)
    x16_d = nc.dram_tensor("x16", [n_blk, WIN, f16_free], F16,
                           kind="ExternalInput")
    x8_d = nc.dram_tensor("x8", [n_blk, WIN, i8_free], I8,
                          kind="ExternalInput")
    b1_d = nc.dram_tensor("b1", [WIN, C_SHARD * WIN], F16, kind="ExternalInput")
    # band2 is used zero-padded to full 128 contraction rows: a [30,128]
    # stationary (partial row-group load) blocks the PE's LDWEIGHTS pull-ahead
    # (HW-probed 318 vs 216ns per-matmul spacing).  Only the 30 nonzero rows
    # are shipped; the zero rows are memset on-chip.
    b2_d = nc.dram_tensor("b2", [OVER, C_SHARD * WIN], F16, kind="ExternalInput")
    y_d = nc.dram_tensor("y", [n_blk, WIN, free], I8, kind="ExternalOutput")

    # dequant chunks: merge adjacent same-engine int8 channels
    def make_chunks(dmap):
        ch = []              # (engine_letter, c0, c1) over int8 channel idx
        for c in range(I8CH):
            if ch and ch[-1][0] == dmap[c]:
                ch[-1][2] = c + 1
            else:
                ch.append([dmap[c], c, c + 1])
        return ch

    deq_chunks = make_chunks(DEQ_MAP)
    # block 0 runs its first int8 groups on the FAST ACT engine so the
    # cold-start dequant chain doesn't stall the PE
    deq_chunks0 = make_chunks(os.environ.get("KERNEL_DEQ_MAP0", "aappaa"))

    n_cg = C_SHARD // CG
    assert F16CH % CG == 0 and I8CH % CG == 0

    with tile.TileContext(nc) as tc:
        with (
            tc.tile_pool(name="const", bufs=1) as const_pool,
            tc.tile_pool(name="xq", bufs=XQ_BUFS) as xq_pool,
            tc.tile_pool(name="xh16", bufs=XH_BUFS) as xh16_pool,
            tc.tile_pool(name="xh8", bufs=XH_BUFS) as xh8_pool,
            tc.tile_pool(name="out", bufs=OUT_BUFS) as out_pool,
            tc.tile_pool(name="psum", bufs=PSUM_BUFS, space="PSUM") as psum_pool,
        ):
            bands_eng = nc.scalar if BANDS_RING == "scalar" else nc.sync
            b1_sb = const_pool.tile([WIN, C_SHARD, WIN], F16)
            bands_eng.dma_start(
                b1_sb[:, :, :], b1_d[:, :].rearrange("p (c m) -> p c m", c=C_SHARD))
            b2_sb = const_pool.tile([WIN, C_SHARD, WIN], F16)
            nc.vector.memset(b2_sb[:, :, :], 0)   # zero rows >= OVER; the DMA
            # below overwrites rows [0, OVER) (WAW-ordered by the scheduler)
            bands_eng.dma_start(
                b2_sb[0:OVER, :, :],
                b2_d[:, :].rearrange("p (c m) -> p c m", c=C_SHARD))

            # PE warm-up burst: throwaway matmuls overlapping the preamble
            # flip the HAM clock gate (1.2 -> 2.4 GHz) before real matmuls
            if N_WARM:
                wf = s_blk * NW
                scratch = const_pool.tile([WIN, max(WIN, wf)], F16)
                nc.gpsimd.memset(scratch[:, :], 0)   # Pool is idle at start
                for i in range(N_WARM):
                    pw = psum_pool.tile([WIN, CG, s_blk, NW], F32, tag="ps",
                                        name=f"warm_{i}")
                    nc.tensor.matmul(pw[:, 0, :, :], scratch[:, 0:WIN],
                                     scratch[:, 0:wf], start=True, stop=True)

            def deq(eng_l, xh8, xq, c0, c1):
                eng = {"d": nc.vector, "a": nc.scalar, "p": nc.gpsimd}[eng_l]
                dst = xh8[:, c0:c1, :, :].rearrange("p c s w -> p (c s w)")
                src = xq[:, c0:c1, :, :].rearrange("p c s w -> p (c s w)")
                if eng_l == "a":
                    eng.copy(dst, src)
                else:
                    eng.tensor_copy(dst, src)

            def issue_input(blk):
                """DMA block blk's inputs; dequant the int8 channels on the
                non-ACT engines.  Called one block AHEAD of the matmuls so
                dequant never gates the PE.  ACT dequant chunks are emitted
                LATER (after ACT's evacs of the current block) so they don't
                delay the WAR-critical evacs on the in-order ACT engine."""
                xq = xq_pool.tile([WIN, I8CH, s_blk, NW], I8, tag="xq",
                                  name=f"xq_{blk}")
                xh16 = xh16_pool.tile([WIN, F16CH, s_blk, NW], F16, tag="xh16",
                                      name=f"xh16_{blk}")
                if blk == 0:
                    # block 0: the whole fp16 load lands first (the per-tile
                    # dep tracker makes the first matmul wait on all of it),
                    # then the int8 load
                    nc.sync.dma_start(
                        xh16[:, :, :, :].rearrange("p c s w -> p (c s w)"),
                        x16_d[blk][:, :])
                    nc.sync.dma_start(
                        xq[:, :, :, :].rearrange("p c s w -> p (c s w)"),
                        x8_d[blk][:, :])
                else:
                    # x8 first: the Pool/ACT dequant chain is longer than
                    # the fp16-direct path, so its input should land first
                    nc.sync.dma_start(
                        xq[:, :, :, :].rearrange("p c s w -> p (c s w)"),
                        x8_d[blk][:, :])
                    nc.sync.dma_start(
                        xh16[:, :, :, :].rearrange("p c s w -> p (c s w)"),
                        x16_d[blk][:, :])
                xh8 = xh8_pool.tile([WIN, I8CH, s_blk, NW], F16, tag="xh8",
                                    name=f"xh8_{blk}")
                for eng_l, c0, c1 in (deq_chunks0 if blk == 0 else deq_chunks):
                    if eng_l != "a":
                        deq(eng_l, xh8, xq, c0, c1)
                return xh16, xh8, xq, blk

            def issue_act_deq(nxt_tiles):
                _, xh8, xq, nblk = nxt_tiles
                for eng_l, c0, c1 in (deq_chunks0 if nblk == 0 else deq_chunks):
                    if eng_l == "a":
                        deq(eng_l, xh8, xq, c0, c1)

            nxt = issue_input(0)
            issue_act_deq(nxt)
            for blk in range(n_blk):
                xh16, xh8 = nxt[0], nxt[1]
                if blk + 1 < n_blk:
                    nxt = issue_input(blk + 1)

                out_t = out_pool.tile([WIN, C_SHARD, s_blk, NW], I8,
                                      tag="out", name=f"out_{blk}")
                # groups 0..F16CH/CG-1 read xh16 (not gated on dequant, so
                # the PE starts immediately); later groups read xh8
                for g in range(n_cg):
                    c0 = g * CG
                    i8_base = g * CG - F16CH
                    ps = psum_pool.tile([WIN, CG, s_blk, NW], F32, tag="ps",
                                        name=f"ps_{blk}_{g}")
                    for j in range(CG):
                        c = c0 + j
                        xh_t, ci = (xh16, c) if c < F16CH else (xh8, i8_base + j)
                        # main band: chunk w taps fully inside window w
                        nc.tensor.matmul(ps[:, j, :, :], b1_sb[:, c, :],
                                         xh_t[:, ci, :, :],
                                         start=True, stop=False)
                        # corner band: chunk w outputs m>=98 borrow rows
                        # [0,30) of window w+1 (chunk NW-1 keeps only m<98;
                        # the rest is sliced off host-side)
                        nc.tensor.matmul(ps[:, j, :, 0:NW - 1], b2_sb[:, c, :],
                                         xh_t[:, ci, :, 1:NW],
                                         start=False, stop=True)
                    # evac: fp32 psum (already y/sy) -> int8 out tile, flat
                    dst = out_t[:, c0:c0 + CG, :, :].rearrange(
                        "p c s w -> p (c s w)")
                    src = ps[:, :, :, :].rearrange("p c s w -> p (c s w)")
                    if EVAC_MAP[g] == "d":
                        nc.vector.tensor_copy(dst, src)
                    else:
                        nc.scalar.copy(dst, src)
                if blk == n_blk - 1:
                    # split the final output DMA so its first half overlaps
                    # the last evacs, shrinking the drain tail
                    half = free // 2
                    nc.scalar.dma_start(
                        y_d[blk][:, 0:half],
                        out_t[:, 0:C_SHARD // 2, :, :].rearrange(
                            "p c s w -> p (c s w)"))
                    nc.scalar.dma_start(
                        y_d[blk][:, half:free],
                        out_t[:, C_SHARD // 2:, :, :].rearrange(
                            "p c s w -> p (c s w)"))
                else:
                    nc.scalar.dma_start(
                        y_d[blk][:, :],
                        out_t[:, :, :, :].rearrange("p c s w -> p (c s w)"))
                # ACT dequant for the NEXT block: after this block's ACT
                # evacs and out-DMA issue, so neither is delayed behind it
                # on the in-order ACT engine
                if blk + 1 < n_blk:
                    issue_act_deq(nxt)
    nc.finalize()
    return nc


def make_kern(attention_weights: np.ndarray, proj_w: np.ndarray,
              proj_b: np.ndarray) -> np.ndarray:
    return (attention_weights.astype(np.float64) @ proj_w.T.astype(np.float64)
            + proj_b.astype(np.float64)).astype(np.float32)          # [C, K]


def make_bands(kern: np.ndarray):
    """kern [C, K] -> b1 [WIN, C, WIN], b2 [WIN, C, WIN] (f32, UNscaled).

    b1[p, c, m] = kern[c, p-m]    for 0 <= p-m < K
    b2[q, c, m] = kern[c, WIN+q-m] for 0 < WIN+q-m < K  (corner taps,
    rows q >= OVER stay zero -- full-height stationary for LDW pipelining)
    """
    b1 = np.zeros((WIN, C, WIN), np.float32)
    m = np.arange(WIN)
    for k in range(K):
        mm = m[m <= WIN - 1 - k]
        b1[mm + k, :, mm] = kern[:, k]
    b2 = np.zeros((WIN, C, WIN), np.float32)
    for k in range(1, K):
        mm = m[m >= WIN - k]
        b2[mm + k - WIN, :, mm] = kern[:, k]
    return b1, b2


def _estimate_sy(x: np.ndarray, kern: np.ndarray) -> float:
    """Sampled estimate of absmax(y): exact conv on every 8th batch,
    inflated by SY_INFLATE to cover the unsampled extreme tail."""
    xs = x[::8].astype(np.float32)                            # [12, L, C]
    lo = L - K + 1
    y = np.zeros((xs.shape[0], lo, C), np.float32)
    for k in range(K):
        y += xs[:, k:k + lo, :] * kern[None, None, :, k]
    return float(np.abs(y).max()) * SY_INFLATE / 127.0


_LAST_SY = [1.0]


def make_in_maps(x: np.ndarray, b1: np.ndarray, b2: np.ndarray,
                 s_blk: int = S_BLK) -> list:
    n_blk = B // s_blk
    x = np.asarray(x, np.float32)
    # which channels ship int8: per core, the LAST I8CH of its 12
    ch = np.arange(C)
    is_i8 = (ch % C_SHARD) >= F16CH
    # per-channel int8 quantization (scale folded into the bands);
    # fp16-direct channels get scale 1
    sxc = np.where(is_i8, np.abs(x).max(axis=(0, 1)) / 127.0, 1.0)    # [C]
    # int8 output scale sy; 1/sy folded into the bands so PSUM holds y/sy
    kern = np.ascontiguousarray(b1[0:K, :, 0].T)              # [C, K] recovered
    sy = _estimate_sy(x, kern)
    _LAST_SY[0] = sy
    xq8 = np.clip(np.rint(x / sxc[None, None, :]), -127, 127).astype(np.int8)
    x16 = x.astype(np.float16)
    # [blk, p, c, s, w] relayout: t = WIN*w + p, b = s_blk*blk + s
    xt8 = xq8.reshape(n_blk, s_blk, NW, WIN, C).transpose(0, 3, 4, 1, 2)
    xt16 = x16.reshape(n_blk, s_blk, NW, WIN, C).transpose(0, 3, 4, 1, 2)
    scale = sxc[None, :, None] / sy
    b1h = (b1 * scale).astype(np.float16)
    b2h = (b2 * scale).astype(np.float16)
    maps = []
    for i in range(N_CORES):
        c0 = i * C_SHARD
        f16_sl = slice(c0, c0 + F16CH)
        i8_sl = slice(c0 + F16CH, c0 + C_SHARD)
        maps.append({
            "x16": np.ascontiguousarray(xt16[:, :, f16_sl]).reshape(
                n_blk, WIN, -1),
            "x8": np.ascontiguousarray(xt8[:, :, i8_sl]).reshape(
                n_blk, WIN, -1),
            "b1": np.ascontiguousarray(b1h[:, c0:c0 + C_SHARD]).reshape(WIN, -1),
            "b2": np.ascontiguousarray(
                b2h[0:OVER, c0:c0 + C_SHARD]).reshape(OVER, -1),
        })
    return maps


def unshard(results, s_blk: int = S_BLK) -> np.ndarray:
    n_blk = B // s_blk
    sy = _LAST_SY[0]
    ys = [np.asarray(r["y"]).reshape(n_blk, WIN, C_SHARD, s_blk, NW)
          for r in results]
    y = np.concatenate(ys, axis=2)                   # [blk, m, C, s, w] int8
    y = y.transpose(0, 3, 4, 1, 2).reshape(B, L, C)[:, :L_OUT, :]
    return np.ascontiguousarray(y.astype(np.float32) * np.float32(sy))


_NC_CACHE: dict = {}


def kernel(x: np.ndarray, attention_weights: np.ndarray,
           proj_w: np.ndarray, proj_b: np.ndarray) -> np.ndarray:
    x = np.asarray(x)
    kern = make_kern(np.asarray(attention_weights), np.asarray(proj_w),
                     np.asarray(proj_b))
    b1, b2 = make_bands(kern)

    if "nc" not in _NC_CACHE:
        _NC_CACHE["nc"] = build_nc()
    nc = _NC_CACHE["nc"]

    in_maps = make_in_maps(x, b1, b2)
    res = run_bass_kernel_spmd(nc, in_maps, core_ids=list(range(N_CORES)))
    return unshard(res.results)


# revision 38
# speedup vs baseline: 1.1437x; 1.1437x over previous
"""Trainium2 Bass kernel for AttentionGuidedConv.

Reference semantics (B=C=96, L=8192, K=31, A=512):
    kernels = attention_weights @ proj_w.T + proj_b          # [96, 31]
    y[b, t, o] = sum_k x[b, t+k, o] * kernels[o, k]          # [96, 8162, 96]

The conv weight depends only on the channel index o, so every batch shares
channel o's kernel.

Strategy (final, HW-measured ~90us vs the 118.7us fp16-wire baseline):
  - The fp16-wire baseline was DMA-byte-bound (37.7MB/core, ~360GB/s
    per-core roofline).  The error metric is max|err|/max|expected|
    (GLOBAL absmax), so uniform int8 quantization is far kinder than
    fp8; int8 needs an on-chip int8->fp16 dequant pass because the PE
    only reads fp32/bf16/fp16/fp8 (no int8 matmul on TRN2).
  - HW-measured engine rates (cost model is optimistic): Pool
    tensor_copy ~2.0ns/free-elem, DVE int8-in ~1.5ns, DVE fp32-in
    ~1.2ns, ACT ~0.54-1.1ns; 4-D strided APs add ~0.5us/instruction
    vs flat [128, N] ones.  Full-int8 input makes dequant + PSUM-evac
    engine time (~85us) exceed the DMA floor it buys (54us), so:
  - MIXED wire: 6 of 12 channels/core ship fp16 straight into SBUF
    (zero dequant), 6 ship int8 (dequant: ch 0-1 Pool, ch 2-5 ACT;
    DVE does none -- it is slowest at int8).  OUTPUT ships int8:
    global y scale sy (host-sampled absmax x1.15), 1/sy folded into
    the band matrices along with the per-channel x scales, so the
    PSUM evac copy converts fp32->int8 directly.  End-to-end rel err
    1.296e-2 on the fixed-seed inputs vs the 2e-2 gate.
  - PSUM tiles are [128, 2ch, 8, 64] (2 banks x 4 bufs = all 8): 4
    accumulating matmuls per tile, one flat evac per tile (DVE groups
    0,1,4,5 / ACT groups 2,3).  4-deep rotation keeps the PE p-state
    at 2.4GHz (215ns per 512-col matmul, measured).
  - Software pipelining: input DMAs + Pool dequant issue one block
    AHEAD of the matmuls; ACT dequant for block b+1 is emitted AFTER
    block b's ACT evacs + out-DMA (in-order engine, so order = fate);
    fp16-direct channel groups matmul first each block; 12 warmup
    matmuls flip the PE clock gate during the DMA preamble; the last
    block's out-DMA is split so the drain tail overlaps the evacs.
  - Channel sharding (12 ch/core), non-overlapping 128-row windows,
    corner-band second matmul (outputs m>=98 borrow rows [0,30) of
    the next window), zero-padded full-height stationary, contiguous
    host-side relayout, sync-ring in / scalar-ring out DMA inherited
    from the baseline (kernel_baseline_v0.py.bak).

Per-core traffic: 14.2MB in (6ch fp16 + 6ch int8 + bands) + 9.44MB out
int8 = ~24MB -> ~64us DMA busy; DVE ~58us, ACT ~50us, Pool ~44us; PE
~66us busy at 2.4GHz -- the per-block critical path is PE (5.2us) +
~0.45us block-boundary resync, plus ~8us start and ~7us drain.
"""

import os

import numpy as np

import concourse.bass as bass
import concourse.bacc as bacc
import concourse.mybir as mybir
import concourse.tile as tile
from concourse.bass_utils import run_bass_kernel_spmd

F32 = mybir.dt.float32
F16 = mybir.dt.float16
I8 = mybir.dt.int8

B, L, C = 96, 8192, 96
K = 31
A = 512
N_CORES = 8
C_SHARD = C // N_CORES          # 12 channels per core
WIN = 128                       # window rows == outputs per chunk (no overlap)
NW = L // WIN                   # 64 windows
OVER = K - 1                    # 30 rows borrowed from the next window
L_OUT = L - K + 1               # 8162
CG = int(os.environ.get("KERNEL_CG", "2"))  # channels per evac chunk & psum tile

S_BLK = int(os.environ.get("KERNEL_S_BLK", "8"))      # batches per block
N_BLK = B // S_BLK
F16CH = int(os.environ.get("KERNEL_F16CH", "6"))      # fp16-direct ch/core
I8CH = C_SHARD - F16CH                                # int8 channels/core
XQ_BUFS = int(os.environ.get("KERNEL_XQ_BUFS", "5"))  # int8 staging tiles
XH_BUFS = int(os.environ.get("KERNEL_XH_BUFS", "5"))  # fp16 input tiles
OUT_BUFS = int(os.environ.get("KERNEL_OUT_BUFS", "5"))
PSUM_BUFS = int(os.environ.get("KERNEL_PSUM_BUFS", "4"))
N_WARM = int(os.environ.get("KERNEL_N_WARM", "12"))    # PE warm-up matmuls
# sync | scalar: ring for the one-shot band-matrix loads
BANDS_RING = os.environ.get("KERNEL_BANDS_RING", "scalar")
# dequant engine per int8 channel (p=Pool, d=DVE, a=ACT), len == I8CH;
# adjacent same-letter channels merge into one instruction
DEQ_MAP = os.environ.get("KERNEL_DEQ_MAP", "ppaaaa")
# evac engine per 3-ch chunk index 0..3    (d=DVE, a=ACT)
EVAC_MAP = os.environ.get("KERNEL_EVAC_MAP", "ddaadd")
# output int8 absmax safety inflation over the sampled estimate
SY_INFLATE = float(os.environ.get("KERNEL_SY_INFLATE", "1.15"))


def build_nc(s_blk: int = S_BLK) -> bass.Bass:
    n_blk = B // s_blk
    free = C_SHARD * s_blk * NW
    f16_free = F16CH * s_blk * NW
    i8_free = I8CH * s_blk * NW
    nc = bacc.Bacc(None, target_bir_lowering=False)
    x16_d = nc.dram_tensor("x16", [n_blk, WIN, f16_free], F16,
                           kind="ExternalInput")
    x8_d = nc.dram_tensor("x8", [n_blk, WIN, i8_free], I8,
                          kind="ExternalInput")
    b1_d = nc.dram_tensor("b1", [WIN, C_SHARD * WIN], F16, kind="ExternalInput")
    # band2 is used zero-padded to full 128 contraction rows: a [30,128]
    # stationary (partial row-group load) blocks the PE's LDWEIGHTS pull-ahead
    # (HW-probed 318 vs 216ns per-matmul spacing).  Only the 30 nonzero rows
    # are shipped; the zero rows are memset on-chip.
    b2_d = nc.dram_tensor("b2", [OVER, C_SHARD * WIN], F16, kind="ExternalInput")
    y_d = nc.dram_tensor("y", [n_blk, WIN, free], I8, kind="ExternalOutput")

    # dequant chunks: merge adjacent same-engine int8 channels
    def make_chunks(dmap):
        ch = []              # (engine_letter, c0, c1) over int8 channel idx
        for c in range(I8CH):
            if ch and ch[-1][0] == dmap[c]:
                ch[-1][2] = c + 1
            else:
                ch.append([dmap[c], c, c + 1])
        return ch

    deq_chunks = make_chunks(DEQ_MAP)
    # block 0 runs its first int8 groups on the FAST ACT engine so the
    # cold-start dequant chain doesn't stall the PE
    deq_chunks0 = make_chunks(os.environ.get("KERNEL_DEQ_MAP0", DEQ_MAP))

    n_cg = C_SHARD // CG
    assert F16CH % CG == 0 and I8CH % CG == 0

    with tile.TileContext(nc) as tc:
        with (
            tc.tile_pool(name="const", bufs=1) as const_pool,
            tc.tile_pool(name="xq", bufs=XQ_BUFS) as xq_pool,
            tc.tile_pool(name="xh16", bufs=XH_BUFS) as xh16_pool,
            tc.tile_pool(name="xh8", bufs=XH_BUFS) as xh8_pool,
            tc.tile_pool(name="out", bufs=OUT_BUFS) as out_pool,
            tc.tile_pool(name="psum", bufs=PSUM_BUFS, space="PSUM") as psum_pool,
        ):
            bands_eng = nc.scalar if BANDS_RING == "scalar" else nc.sync
            b1_sb = const_pool.tile([WIN, C_SHARD, WIN], F16)
            bands_eng.dma_start(
                b1_sb[:, :, :], b1_d[:, :].rearrange("p (c m) -> p c m", c=C_SHARD))
            b2_sb = const_pool.tile([WIN, C_SHARD, WIN], F16)
            nc.vector.memset(b2_sb[:, :, :], 0)   # zero rows >= OVER; the DMA
            # below overwrites rows [0, OVER) (WAW-ordered by the scheduler)
            bands_eng.dma_start(
                b2_sb[0:OVER, :, :],
                b2_d[:, :].rearrange("p (c m) -> p c m", c=C_SHARD))

            # PE warm-up burst: throwaway matmuls overlapping the preamble
            # flip the HAM clock gate (1.2 -> 2.4 GHz) before real matmuls
            if N_WARM:
                wf = s_blk * NW
                scratch = const_pool.tile([WIN, max(WIN, wf)], F16)
                nc.gpsimd.memset(scratch[:, :], 0)   # Pool is idle at start
                for i in range(N_WARM):
                    pw = psum_pool.tile([WIN, CG, s_blk, NW], F32, tag="ps",
                                        name=f"warm_{i}")
                    nc.tensor.matmul(pw[:, 0, :, :], scratch[:, 0:WIN],
                                     scratch[:, 0:wf], start=True, stop=True)

            def deq(eng_l, xh8, xq, c0, c1):
                eng = {"d": nc.vector, "a": nc.scalar, "p": nc.gpsimd}[eng_l]
                dst = xh8[:, c0:c1, :, :].rearrange("p c s w -> p (c s w)")
                src = xq[:, c0:c1, :, :].rearrange("p c s w -> p (c s w)")
                if eng_l == "a":
                    eng.copy(dst, src)
                else:
                    eng.tensor_copy(dst, src)

            def issue_input(blk):
                """DMA block blk's inputs; dequant the int8 channels on the
                non-ACT engines.  Called one block AHEAD of the matmuls so
                dequant never gates the PE.  ACT dequant chunks are emitted
                LATER (after ACT's evacs of the current block) so they don't
                delay the WAR-critical evacs on the in-order ACT engine."""
                xq = xq_pool.tile([WIN, I8CH, s_blk, NW], I8, tag="xq",
                                  name=f"xq_{blk}")
                xh16 = xh16_pool.tile([WIN, F16CH, s_blk, NW], F16, tag="xh16",
                                      name=f"xh16_{blk}")
                if blk == 0:
                    # block 0: first fp16 channels land first, then the int8
                    # load (dequant lead time), then the rest
                    hc = F16CH // 2
                    hf = f16_free // 2
                    nc.sync.dma_start(
                        xh16[:, 0:hc, :, :].rearrange("p c s w -> p (c s w)"),
                        x16_d[blk][:, 0:hf])
                    nc.sync.dma_start(
                        xq[:, :, :, :].rearrange("p c s w -> p (c s w)"),
                        x8_d[blk][:, :])
                    nc.sync.dma_start(
                        xh16[:, hc:, :, :].rearrange("p c s w -> p (c s w)"),
                        x16_d[blk][:, hf:])
                else:
                    # x8 first: the Pool/ACT dequant chain is longer than
                    # the fp16-direct path, so its input should land first
                    nc.sync.dma_start(
                        xq[:, :, :, :].rearrange("p c s w -> p (c s w)"),
                        x8_d[blk][:, :])
                    nc.sync.dma_start(
                        xh16[:, :, :, :].rearrange("p c s w -> p (c s w)"),
                        x16_d[blk][:, :])
                xh8 = xh8_pool.tile([WIN, I8CH, s_blk, NW], F16, tag="xh8",
                                    name=f"xh8_{blk}")
                for eng_l, c0, c1 in (deq_chunks0 if blk == 0 else deq_chunks):
                    if eng_l != "a":
                        deq(eng_l, xh8, xq, c0, c1)
                return xh16, xh8, xq, blk

            def issue_act_deq(nxt_tiles):
                _, xh8, xq, nblk = nxt_tiles
                for eng_l, c0, c1 in (deq_chunks0 if nblk == 0 else deq_chunks):
                    if eng_l == "a":
                        deq(eng_l, xh8, xq, c0, c1)

            nxt = issue_input(0)
            issue_act_deq(nxt)
            for blk in range(n_blk):
                xh16, xh8 = nxt[0], nxt[1]
                if blk + 1 < n_blk:
                    nxt = issue_input(blk + 1)

                out_t = out_pool.tile([WIN, C_SHARD, s_blk, NW], I8,
                                      tag="out", name=f"out_{blk}")
                # groups 0..F16CH/CG-1 read xh16 (not gated on dequant, so
                # the PE starts immediately); later groups read xh8
                for g in range(n_cg):
                    c0 = g * CG
                    i8_base = g * CG - F16CH
                    ps = psum_pool.tile([WIN, CG, s_blk, NW], F32, tag="ps",
                                        name=f"ps_{blk}_{g}")
                    for j in range(CG):
                        c = c0 + j
                        xh_t, ci = (xh16, c) if c < F16CH else (xh8, i8_base + j)
                        # main band: chunk w taps fully inside window w
                        nc.tensor.matmul(ps[:, j, :, :], b1_sb[:, c, :],
                                         xh_t[:, ci, :, :],
                                         start=True, stop=False)
                        # corner band: chunk w outputs m>=98 borrow rows
                        # [0,30) of window w+1 (chunk NW-1 keeps only m<98;
                        # the rest is sliced off host-side)
                        nc.tensor.matmul(ps[:, j, :, 0:NW - 1], b2_sb[:, c, :],
                                         xh_t[:, ci, :, 1:NW],
                                         start=False, stop=True)
                    # evac: fp32 psum (already y/sy) -> int8 out tile, flat
                    dst = out_t[:, c0:c0 + CG, :, :].rearrange(
                        "p c s w -> p (c s w)")
                    src = ps[:, :, :, :].rearrange("p c s w -> p (c s w)")
                    if EVAC_MAP[g] == "d":
                        nc.vector.tensor_copy(dst, src)
                    else:
                        nc.scalar.copy(dst, src)
                if blk == n_blk - 1:
                    # split the final output DMA so its first half overlaps
                    # the last evacs, shrinking the drain tail
                    half = free // 2
                    nc.scalar.dma_start(
                        y_d[blk][:, 0:half],
                        out_t[:, 0:C_SHARD // 2, :, :].rearrange(
                            "p c s w -> p (c s w)"))
                    nc.scalar.dma_start(
                        y_d[blk][:, half:free],
                        out_t[:, C_SHARD // 2:, :, :].rearrange(
                            "p c s w -> p (c s w)"))
                else:
                    nc.scalar.dma_start(
                        y_d[blk][:, :],
                        out_t[:, :, :, :].rearrange("p c s w -> p (c s w)"))
                # ACT dequant for the NEXT block: after this block's ACT
                # evacs and out-DMA issue, so neither is delayed behind it
                # on the in-order ACT engine
                if blk + 1 < n_blk:
                    issue_act_deq(nxt)
    nc.finalize()
    return nc


def make_kern(attention_weights: np.ndarray, proj_w: np.ndarray,
              proj_b: np.ndarray) -> np.ndarray:
    return (attention_weights.astype(np.float64) @ proj_w.T.astype(np.float64)
            + proj_b.astype(np.float64)).astype(np.float32)          # [C, K]


def make_bands(kern: np.ndarray):
    """kern [C, K] -> b1 [WIN, C, WIN], b2 [WIN, C, WIN] (f32, UNscaled).

    b1[p, c, m] = kern[c, p-m]    for 0 <= p-m < K
    b2[q, c, m] = kern[c, WIN+q-m] for 0 < WIN+q-m < K  (corner taps,
    rows q >= OVER stay zero -- full-height stationary for LDW pipelining)
    """
    b1 = np.zeros((WIN, C, WIN), np.float32)
    m = np.arange(WIN)
    for k in range(K):
        mm = m[m <= WIN - 1 - k]
        b1[mm + k, :, mm] = kern[:, k]
    b2 = np.zeros((WIN, C, WIN), np.float32)
    for k in range(1, K):
        mm = m[m >= WIN - k]
        b2[mm + k - WIN, :, mm] = kern[:, k]
    return b1, b2


def _estimate_sy(x: np.ndarray, kern: np.ndarray) -> float:
    """Sampled estimate of absmax(y): exact conv on every 8th batch,
    inflated by SY_INFLATE to cover the unsampled extreme tail."""
    xs = x[::8].astype(np.float32)                            # [12, L, C]
    lo = L - K + 1
    y = np.zeros((xs.shape[0], lo, C), np.float32)
    for k in range(K):
        y += xs[:, k:k + lo, :] * kern[None, None, :, k]
    return float(np.abs(y).max()) * SY_INFLATE / 127.0


_LAST_SY = [1.0]


def make_in_maps(x: np.ndarray, b1: np.ndarray, b2: np.ndarray,
                 s_blk: int = S_BLK) -> list:
    n_blk = B // s_blk
    x = np.asarray(x, np.float32)
    # which channels ship int8: per core, the LAST I8CH of its 12
    ch = np.arange(C)
    is_i8 = (ch % C_SHARD) >= F16CH
    # per-channel int8 quantization (scale folded into the bands);
    # fp16-direct channels get scale 1
    sxc = np.where(is_i8, np.abs(x).max(axis=(0, 1)) / 127.0, 1.0)    # [C]
    # int8 output scale sy; 1/sy folded into the bands so PSUM holds y/sy
    kern = np.ascontiguousarray(b1[0:K, :, 0].T)              # [C, K] recovered
    sy = _estimate_sy(x, kern)
    _LAST_SY[0] = sy
    xq8 = np.clip(np.rint(x / sxc[None, None, :]), -127, 127).astype(np.int8)
    x16 = x.astype(np.float16)
    # [blk, p, c, s, w] relayout: t = WIN*w + p, b = s_blk*blk + s
    xt8 = xq8.reshape(n_blk, s_blk, NW, WIN, C).transpose(0, 3, 4, 1, 2)
    xt16 = x16.reshape(n_blk, s_blk, NW, WIN, C).transpose(0, 3, 4, 1, 2)
    scale = sxc[None, :, None] / sy
    b1h = (b1 * scale).astype(np.float16)
    b2h = (b2 * scale).astype(np.float16)
    maps = []
    for i in range(N_CORES):
        c0 = i * C_SHARD
        f16_sl = slice(c0, c0 + F16CH)
        i8_sl = slice(c0 + F16CH, c0 + C_SHARD)
        maps.append({
            "x16": np.ascontiguousarray(xt16[:, :, f16_sl]).reshape(
                n_blk, WIN, -1),
            "x8": np.ascontiguousarray(xt8[:, :, i8_sl]).reshape(
                n_blk, WIN, -1),
            "b1": np.ascontiguousarray(b1h[:, c0:c0 + C_SHARD]).reshape(WIN, -1),
            "b2": np.ascontiguousarray(
                b2h[0:OVER, c0:c0 + C_SHARD]).reshape(OVER, -1),
        })
    return maps


def unshard(results, s_blk: int = S_BLK) -> np.ndarray:
    n_blk = B // s_blk
    sy = _LAST_SY[0]
    ys = [np.asarray(r["y"]).reshape(n_blk, WIN, C_SHARD, s_blk, NW)
          for r in results]
    y = np.concatenate(ys, axis=2)                   # [blk, m, C, s, w] int8
    y = y.transpose(0, 3, 4, 1, 2).reshape(B, L, C)[:, :L_OUT, :]
    return np.ascontiguousarray(y.astype(np.float32) * np.float32(sy))


_NC_CACHE: dict = {}


def kernel(x: np.ndarray, attention_weights: np.ndarray,
           proj_w: np.ndarray, proj_b: np.ndarray) -> np.ndarray:
    x = np.asarray(x)
    kern = make_kern(np.asarray(attention_weights), np.asarray(proj_w),
                     np.asarray(proj_b))
    b1, b2 = make_bands(kern)

    if "nc" not in _NC_CACHE:
        _NC_CACHE["nc"] = build_nc()
    nc = _NC_CACHE["nc"]

    in_maps = make_in_maps(x, b1, b2)
    res = run_bass_kernel_spmd(nc, in_maps, core_ids=list(range(N_CORES)))
    return unshard(res.results)
